# revision 58
# baseline (speedup 1.0000x reference)
"""Dinov3 ViT attention kernel for Trainium2 (8 NeuronCores, data-parallel over batch).

Per core: 2 batch items. hidden_states [2*1029, 1024] in, out [2*1029, 1024] f32.

v2: QKV projections run as fp8e4m3 DoubleRow matmuls with hi+lo error
compensation (host splits X and the QKV weights into e4m3 hi/lo pairs;
weights pre-scaled by 32 so the lo half stays in e4m3 normal range, undone
in the PSUM-evacuation op). Each 128-feature k-tile pair contracts 256 deep
at 0.5 cycles/row, cutting projection PE time 25% at ~bf16 accuracy.
PSUM evacuation (bias/scale) moves to the Pool engine to unload DVE.

Per item pipeline (PE-dense, interleaved with ACT-bound attention):
  X-prep (strided fp8 hi/lo DMAs) ->
  V-proj chunks 0,1 (heads 0..7) ->
  for mo in 0..7:  # one 128-feature tile = head pair (2mo, 2mo+1)
      Q-proj(mo) + bias, K-proj(mo), RoPE(mo) on DVE,
      attention for heads 2mo, 2mo+1:
        S^T per key-tile (K=64 matmul), exp on ScalarE (scale=1/8, no max:
        |scores| < ~7), AV matmul with ones-augmented V (row 64 = softmax sums),
        5-query tail batched into one [128,45] PSUM bank + single exp per head,
        normalize via DVE reciprocal + gpsimd partition_broadcast.
      (V-proj chunk 2 at mo=2, chunk 3 at mo=3)
  output projection Y = (AttnOut^T)^T Wo + bo -> DMA f32.
"""
import sys
import time

sys.path.insert(0, "/opt/trn_rl_repo")

import ml_dtypes
import numpy as np

import concourse.bacc as bacc
import concourse.mybir as mybir
import concourse.tile as tile

f32 = mybir.dt.float32
bf16 = mybir.dt.bfloat16
fp8 = mybir.dt.float8e4
FP = mybir.ActivationFunctionType
ADD = mybir.AluOpType.add
MUL = mybir.AluOpType.mult
SUB = mybir.AluOpType.subtract
DR = mybir.MatmulPerfMode.DoubleRow

H = 1024
NH = 16
HD = 64
T = 1029
NPREF = 5
PATCH = 1024
B = 16
NCORES = 8
BPC = B // NCORES          # batch items per core
KO = H // 128              # 8 feature k-tiles
TOK = BPC * T              # tokens per core (2058)
SCALE = 1.0 / float(np.sqrt(HD))
WSC = 32.0                 # host pre-scale on fp8-split qkv weights
IWSC = float(1.0 / WSC)

TPAD = 1040                # dual-fp8 ldweights needs 16B-multiple pair stride
TOK_TILES = [(i * 128, min(128, T - i * 128)) for i in range((T + 127) // 128)]
NJT = len(TOK_TILES)
QCHUNKS = [(0, 512), (512, 512)]
QTAIL = (1024, T - 1024)               # 5 queries -> batched-exp path
# DoubleRow moving chunks: rhs free = 2*qw <= 512 -> qw <= 256
DRQ_CHUNKS = [(0, 256), (256, 256), (512, 256), (768, 256), (1024, 5)]
VN_CHUNKS = [(0, 256), (256, 256), (512, 256), (768, 256)]  # V out features


def build():
    nc = bacc.Bacc(None, target_bir_lowering=False)
    hs8_d = nc.dram_tensor("hs8", [H, TOK], fp8, kind="ExternalInput")
    hsr_d = nc.dram_tensor("hsr", [H, TOK], fp8, kind="ExternalInput")
    # host-prepared rope tables: cosT2[d, t] = cos[t, d%64] duplicated;
    # sinT2sw = swapped/sign-flipped sin layout (see emit_rope_t)
    cosT2_d = nc.dram_tensor("cosT2", [128, PATCH], bf16, kind="ExternalInput")
    sinT2n_d = nc.dram_tensor("sinT2n", [128, PATCH], bf16,
                              kind="ExternalInput")
    w_d = {}
    for wn in ("wq", "wk", "wv"):
        w_d[wn + "8"] = nc.dram_tensor(wn + "8", [H, H], fp8,
                                       kind="ExternalInput")
        w_d[wn + "r"] = nc.dram_tensor(wn + "r", [H, H], fp8,
                                       kind="ExternalInput")
    w_d["wo"] = nc.dram_tensor("wo", [H, H], bf16, kind="ExternalInput")
    b_d = {"bq": nc.dram_tensor("bq", [H], f32, kind="ExternalInput"),
           "bv": nc.dram_tensor("bv", [H], bf16, kind="ExternalInput"),
           "bo": nc.dram_tensor("bo", [H], bf16, kind="ExternalInput")}
    out_d = nc.dram_tensor("out", [TOK, H], f32, kind="ExternalOutput")

    with tile.TileContext(nc) as tc:
        with (
            tc.tile_pool(name="const", bufs=1) as cpool,
            tc.tile_pool(name="item", bufs=1) as ipool,
            tc.tile_pool(name="ao", bufs=2) as aopool,
            tc.tile_pool(name="work", bufs=3) as wpool,
            tc.tile_pool(name="rope", bufs=1) as rpool,
            tc.tile_pool(name="stack", bufs=2) as spool,
            tc.tile_pool(name="stack1", bufs=1) as spool1,
            tc.tile_pool(name="attn", bufs=3) as apool,
            tc.tile_pool(name="ypool", bufs=2) as ypool,
            tc.tile_pool(name="attn2", bufs=1) as apool2,
            tc.tile_pool(name="ps_s", bufs=2, space="PSUM") as ps_s,
            tc.tile_pool(name="ps_o", bufs=1, space="PSUM") as ps_o,
            tc.tile_pool(name="ps_w", bufs=2, space="PSUM") as ps_w,
        ):
            # --- X-prep: hs pre-split to fp8 hi/lo, feature-major on host ---
            hs8_r = hs8_d.rearrange("(o p) t -> p o t", p=128)
            hsr_r = hsr_d.rearrange("(o p) t -> p o t", p=128)

            def emit_xprep_full(bi, X8, XR):
                nc.sync.dma_start(X8[:, :, 0:T],
                                  hs8_r[:, :, bi * T: bi * T + T])
                nc.sync.dma_start(XR[:, :, 0:T],
                                  hsr_r[:, :, bi * T: bi * T + T])

            X8_0 = ipool.tile([128, KO, TPAD], fp8, tag="X8", name="X8_0")
            XR_0 = ipool.tile([128, KO, TPAD], fp8, tag="XR", name="XR_0")
            emit_xprep_full(0, X8_0, XR_0)

            # --- rope tables straight from host (ACT hwdge queue: keep the
            # SP queue free for the X stream) ---
            cosT2 = cpool.tile([128, PATCH], bf16)
            sinT2n = cpool.tile([128, PATCH], bf16)
            nc.scalar.dma_start(cosT2[:], cosT2_d[:])
            nc.scalar.dma_start(sinT2n[:], sinT2n_d[:])

            # --- biases, weights (single strided DMA each, ACT queue,
            # ordered by first use: V-proj then Q/K-proj then out-proj) ---
            bq_sb = cpool.tile([128, KO], f32)
            nc.scalar.dma_start(bq_sb[:],
                                b_d["bq"].rearrange("(o p) -> p o", p=128))
            bv_bc = cpool.tile([128, H], bf16)
            nc.scalar.dma_start(bv_bc[:],
                                b_d["bv"][None, :].to_broadcast((128, H)))
            bo_row = cpool.tile([1, H], bf16)
            nc.scalar.dma_start(bo_row[:], b_d["bo"][None, :])
            ones_row = cpool.tile([1, 128], bf16)
            nc.vector.memset(ones_row[:], 1.0)

            wb = {}
            for wn in ("wq8", "wqr", "wk8", "wkr", "wv8", "wvr"):
                wb[wn] = cpool.tile([128, KO, H], fp8, tag=f"wb_{wn}",
                                    name=f"wb_{wn}")
            wb["wo"] = cpool.tile([128, KO, H], bf16, tag="wb_wo", name="wb_wo")
            for wn in ("wv8", "wvr", "wq8", "wqr", "wk8", "wkr", "wo"):
                nc.scalar.dma_start(
                    wb[wn][:], w_d[wn].rearrange("(o p) n -> p o n", p=128))

            # ---------------- per batch item ----------------
            def make_item(bi, X8, XR):
                tok0 = bi * T
                QT = ipool.tile([128, KO, T], bf16, tag="QT", name=f"QT_{bi}")
                KT = ipool.tile([128, KO, T], bf16, tag="KT", name=f"KT_{bi}")
                Vst = ipool.tile([128, NJT, NH, HD + 1], bf16, tag="Vst",
                                 name=f"Vst_{bi}")
                AOT = aopool.tile([128, KO, T], bf16, tag="AOT", name=f"AOT_{bi}")

                def emit_vinit():
                    nc.vector.memset(Vst[:, :, :, HD:HD + 1], 1.0)

                def emit_vproj_t(ci, ti):
                    n0, nw = VN_CHUNKS[ci]
                    t0, tw = TOK_TILES[ti]
                    pm = ps_w.tile([128, 512], f32, tag="ps_w",
                                   name=f"pmv_{bi}_{ci}_{ti}")
                    # 3-term compensated DR: X8*W8, X8*Wr, Xr*W8
                    steps = ([(X8, "wv8")] * 4 + [(X8, "wvr")] * 4
                             + [(XR, "wv8")] * 4)
                    for si, (xt, wn) in enumerate(steps):
                        j = 2 * (si % 4)
                        nc.tensor.matmul(
                            pm[:tw, :nw],
                            xt[:, j:j + 2, t0:t0 + tw],
                            wb[wn][:, j:j + 2, n0:n0 + nw],
                            start=(si == 0), stop=(si == len(steps) - 1),
                            perf_mode=DR)
                    nc.vector.scalar_tensor_tensor(
                        Vst[:tw, ti, n0 // HD:(n0 + nw) // HD, 0:HD],
                        pm[:tw, :nw], IWSC, bv_bc[:tw, n0:n0 + nw],
                        op0=MUL, op1=ADD)

                def emit_qkproj_g(mo, which, ci):
                    dst, w8n, wrn, bias = (
                        (QT, "wq8", "wqr", True), (KT, "wk8", "wkr", False)
                    )[which]
                    q0, qw = DRQ_CHUNKS[ci]
                    pm = ps_w.tile([128, 512], f32, tag="ps_w",
                                   name=f"pm_{bi}_{w8n}_{mo}_{q0}")
                    steps = ([(X8, w8n)] * 4 + [(XR, w8n)] * 4
                             + [(X8, wrn)] * 4)
                    for si, (xt, wn) in enumerate(steps):
                        j = 2 * (si % 4)
                        nc.tensor.matmul(
                            pm[:, :qw],
                            wb[wn][:, j:j + 2, mo * 128:(mo + 1) * 128],
                            xt[:, j:j + 2, q0:q0 + qw],
                            start=(si == 0), stop=(si == len(steps) - 1),
                            perf_mode=DR)
                    if bias:
                        nc.vector.scalar_tensor_tensor(
                            dst[:, mo, q0:q0 + qw], pm[:, :qw], IWSC,
                            bq_sb[:, mo:mo + 1].to_broadcast((128, qw)),
                            op0=MUL, op1=ADD)
                    else:
                        nc.vector.tensor_scalar_mul(
                            dst[:, mo, q0:q0 + qw], pm[:, :qw], IWSC)

                stacks = {}

                def emit_stack(mo):
                    # fp8 hi/lo operand stacks for DoubleRow scores:
                    #   SQ_h = [[q8;q8],[qr;qr]]  (rhs slices)
                    #   SK_h = [k8;kr]            (lhsT, used for both slices)
                    # S = (k8+kr)^T (q8+qr): full hi/lo product, ~bf16 accuracy
                    qsp = spool1.tile([128, 2, TPAD], fp8, tag="qsp")
                    nc.scalar.copy(qsp[:, 0, 0:T], QT[:, mo, :])
                    nc.vector.tensor_tensor(qsp[:, 1, 0:T], QT[:, mo, :],
                                            qsp[:, 0, 0:T], SUB)
                    ksp = spool1.tile([128, 2, TPAD], fp8, tag="ksp")
                    nc.scalar.copy(ksp[:, 0, 0:T], KT[:, mo, :])
                    nc.vector.tensor_tensor(ksp[:, 1, 0:T], KT[:, mo, :],
                                            ksp[:, 0, 0:T], SUB)
                    tiles = []
                    for half in range(2):
                        ph = half * 64
                        sq = spool.tile([128, 2, TPAD], fp8, tag=f"sq{half}")
                        sk = spool.tile([128, TPAD], fp8, tag=f"sk{half}")
                        # hi/lo pairs travel together: one DMA per 64-row dst
                        for dst_ph in (0, 64):
                            nc.sync.dma_start(sq[dst_ph:dst_ph + 64, :, :],
                                              qsp[ph:ph + 64, :, :])
                        nc.scalar.dma_start(sk[0:64, 0:T],
                                            ksp[ph:ph + 64, 0, 0:T])
                        nc.scalar.dma_start(sk[64:128, 0:T],
                                            ksp[ph:ph + 64, 1, 0:T])
                        tiles.append((sq, sk))
                    stacks[mo] = tiles

                def emit_rope_t(mo, which):
                    tgt = (QT, KT)[which]
                    src = tgt[:, mo, NPREF:T]
                    t1 = rpool.tile([128, PATCH], bf16, tag="rope1")
                    nc.vector.tensor_tensor(t1[:], src, cosT2[:], MUL)
                    t2 = rpool.tile([128, PATCH], bf16, tag="rope2")
                    for (o, sp) in ((0, 32), (32, 0), (64, 96), (96, 64)):
                        nc.vector.tensor_tensor(
                            t2[o:o + 32, :], tgt[sp:sp + 32, mo, NPREF:T],
                            sinT2n[sp:sp + 32, :], MUL)
                    nc.vector.tensor_tensor(src, t1[:], t2[:], ADD)

                def emit_attn(h, pump=None):
                    ph = (h % 2) * 64
                    kq = h // 2
                    po = ps_o.tile([128, 1024], f32, tag="ps_o",
                                   name=f"po_{bi}_{h}")

                    sq, sk = stacks[kq][h % 2]

                    def s_mm(ji):
                        j0, jw = TOK_TILES[ji]
                        pss = ps_s.tile([128, 1024], f32, tag="ps_s")
                        for q0 in range(0, 1024, 256):
                            nc.tensor.matmul(
                                pss[:jw, q0:q0 + 256],
                                sk[:, None, j0:j0 + jw].to_broadcast(
                                    (128, 2, jw)),
                                sq[:, :, q0:q0 + 256],
                                start=True, stop=True, perf_mode=DR)
                        return pss

                    # software pipeline: S(ji+1) and pump work sit between
                    # exp(ji) and AV(ji) in the in-order PE stream, so the PE
                    # computes next-tile scores while ACT runs exp(ji)
                    pss = s_mm(0)
                    for ji, (j0, jw) in enumerate(TOK_TILES):
                        first, last = ji == 0, ji == NJT - 1
                        es = apool.tile([128, 1024], bf16, tag="expS")
                        nc.scalar.activation(es[:jw, :], pss[:jw, :],
                                             FP.Exp, scale=SCALE)
                        if not last:
                            pss = s_mm(ji + 1)
                        if pump is not None:
                            pump()
                        for q0, qw in QCHUNKS:
                            nc.tensor.matmul(
                                po[:HD + 1, q0:q0 + qw],
                                Vst[:jw, ji, h, :],
                                es[:jw, q0:q0 + qw],
                                start=first, stop=last)
                    for qi, (q0, qw) in enumerate(QCHUNKS):
                        rc = apool2.tile([1, 512], f32, tag="recip")
                        nc.vector.reciprocal(rc[0:1, :qw], po[64:65, q0:q0 + qw])
                        rb = apool2.tile([64, 512], f32, tag="recipB")
                        nc.gpsimd.partition_broadcast(rb[:, :qw], rc[0:1, :qw])
                        nc.vector.tensor_tensor(
                            AOT[ph:ph + 64, kq, q0:q0 + qw],
                            po[0:64, q0:q0 + qw], rb[:, :qw], MUL)

                def emit_tail():
                    # 5-query tail for all 16 heads, batched: S packed into one
                    # ps_s slot (heads 0..10 bank A, 11..15 bank B), two exps,
                    # AV accumulated per head into one ps_o slot.
                    qt0, qtw = QTAIL
                    pst = ps_s.tile([128, 1024], f32, tag="ps_s",
                                    name=f"pst_{bi}")
                    nc.vector.memset(pst[:], 0.0)

                    def tcol(h):
                        return (h * qtw * NJT if h <= 10
                                else 512 + (h - 11) * qtw * NJT)

                    for h in range(NH):
                        ph = (h % 2) * 64
                        kq = h // 2
                        for ji, (j0, jw) in enumerate(TOK_TILES):
                            nc.tensor.matmul(
                                pst[:jw,
                                    tcol(h) + ji * qtw: tcol(h) + (ji + 1) * qtw],
                                KT[ph:ph + 64, kq, j0:j0 + jw],
                                QT[ph:ph + 64, kq, qt0:qt0 + qtw],
                                start=True, stop=True)
                    est = apool.tile([128, 1024], bf16, tag="expS",
                                     name=f"est_{bi}")
                    nc.scalar.activation(est[:, 0:495], pst[:, 0:495],
                                         FP.Exp, scale=SCALE)
                    nc.scalar.activation(est[:, 512:737], pst[:, 512:737],
                                         FP.Exp, scale=SCALE)
                    pot = ps_o.tile([128, 1024], f32, tag="ps_o",
                                    name=f"pot_{bi}")
                    for h in range(NH):
                        for ji, (j0, jw) in enumerate(TOK_TILES):
                            nc.tensor.matmul(
                                pot[:HD + 1, h * qtw:(h + 1) * qtw],
                                Vst[:jw, ji, h, :],
                                est[0:jw,
                                    tcol(h) + ji * qtw: tcol(h) + (ji + 1) * qtw],
                                start=(ji == 0), stop=(ji == NJT - 1))
                    rc = apool2.tile([1, 512], f32, tag="recip")
                    nc.vector.reciprocal(rc[0:1, :NH * qtw],
                                         pot[64:65, :NH * qtw])
                    rb = apool2.tile([64, 512], f32, tag="recipB")
                    nc.gpsimd.partition_broadcast(rb[:, :NH * qtw],
                                                  rc[0:1, :NH * qtw])
                    for h in range(NH):
                        nc.vector.tensor_tensor(
                            AOT[(h % 2) * 64:(h % 2) * 64 + 64, h // 2,
                                qt0:qt0 + qtw],
                            pot[0:64, h * qtw:(h + 1) * qtw],
                            rb[:, h * qtw:(h + 1) * qtw], MUL)

                def emit_outproj_g(ti, nci):
                    t0, tw = TOK_TILES[ti]
                    n0, nw = nci * 512, 512
                    pm = ps_w.tile([128, 512], f32, tag="ps_w",
                                   name=f"pmo_{bi}_{ti}_{n0}")
                    for ko in range(KO):
                        nc.tensor.matmul(
                            pm[:tw, :nw],
                            AOT[:, ko, t0:t0 + tw],
                            wb["wo"][:, ko, n0:n0 + nw],
                            start=(ko == 0), stop=False)
                    # K=1 ones-row matmul adds the bias; evac on ACT (the
                    # drain phase is DVE-bound, ACT idles there)
                    nc.tensor.matmul(
                        pm[:tw, :nw], ones_row[0:1, 0:tw],
                        bo_row[0:1, n0:n0 + nw], start=False, stop=True)
                    y = ypool.tile([128, 512], f32, tag="y")
                    nc.scalar.copy(y[:tw, :nw], pm[:tw, :nw])
                    nc.sync.dma_start(
                        out_d[tok0 + t0: tok0 + t0 + tw, n0:n0 + nw],
                        y[:tw, :nw])

                def emit_outproj(skip=()):
                    for ti in range(NJT):
                        for nci in range(2):
                            if (ti, nci) not in skip:
                                emit_outproj_g(ti, nci)

                def emit_blocks(extra=None):
                    # two queues: crit (next mo's q/k proj + rope — MUST be
                    # emitted within this mo's 18 pump slots: program order is
                    # semantics) and bulk (V chunks, deferred out-proj, xprep
                    # — emitted whenever slots are free, may spill)
                    crit = []
                    bulk = []

                    def pump():
                        if crit:
                            crit.pop(0)()
                        elif bulk:
                            bulk.pop(0)()

                    for mo in range(KO):
                        if mo < KO - 1:
                            crit.extend(
                                (lambda mo=mo, which=which, ci=ci:
                                 emit_qkproj_g(mo + 1, which, ci))
                                for which in range(2)
                                for ci in range(len(DRQ_CHUNKS)))
                            crit.append(lambda mo=mo: emit_rope_t(mo + 1, 0))
                            crit.append(lambda mo=mo: emit_rope_t(mo + 1, 1))
                            crit.append(lambda mo=mo: emit_stack(mo + 1))
                        if mo == 0:
                            bulk.extend(
                                (lambda ti=ti: emit_vproj_t(2, ti))
                                for ti in range(NJT))
                        if mo == 1:
                            bulk.extend(
                                (lambda ti=ti: emit_vproj_t(3, ti))
                                for ti in range(NJT))
                        if extra and mo in extra:
                            bulk.extend(extra[mo])
                        emit_attn(2 * mo, pump)
                        emit_attn(2 * mo + 1, pump)
                    while crit or bulk:
                        pump()

                def emit_head():
                    emit_vinit()
                    for ci in range(2):
                        for ti in range(NJT):
                            emit_vproj_t(ci, ti)
                    for which in range(2):
                        for ci in range(len(DRQ_CHUNKS)):
                            emit_qkproj_g(0, which, ci)
                    emit_rope_t(0, 0)
                    emit_rope_t(0, 1)
                    emit_stack(0)

                return {
                    "head": emit_head, "blocks": emit_blocks,
                    "tail": emit_tail, "outproj": emit_outproj,
                    "outproj_g": emit_outproj_g,
                }

            it0 = make_item(0, X8_0, XR_0)
            it0["head"]()
            X8_1 = ipool.tile([128, KO, TPAD], fp8, tag="X8", name="X8_1")
            XR_1 = ipool.tile([128, KO, TPAD], fp8, tag="XR", name="XR_1")
            it0["blocks"](extra={6: [lambda: emit_xprep_full(1, X8_1, XR_1)]})
            it0["tail"]()
            it1 = make_item(1, X8_1, XR_1)
            it1["head"]()              # runs during item0 out-proj
            # spread most of item0's out-proj through item1's attention as
            # extra pump work (2 chunks per mo)
            defer = [(ti, nci) for ti in range(1, NJT) for nci in range(2)]
            it0["outproj"](skip=defer)
            dthunks = [(lambda ti=ti, nci=nci: it0["outproj_g"](ti, nci))
                       for (ti, nci) in defer]
            extra = {mo: dthunks[mo * 2: mo * 2 + 2] for mo in range(7)}
            extra[7] = dthunks[14:]
            it1["blocks"](extra=extra)
            it1["tail"]()
            it1["outproj"]()

    nc.compile()
    return nc


_NC_CACHE = []
_LAST_RESULT = []


def _e4m3(x):
    return np.ascontiguousarray(x.astype(ml_dtypes.float8_e4m3))


def _split8(x):
    """x (f32) -> (hi, lo) e4m3 pair with hi + lo ~ x."""
    hi = x.astype(ml_dtypes.float8_e4m3)
    lo = (x - hi.astype(np.float32)).astype(ml_dtypes.float8_e4m3)
    return np.ascontiguousarray(hi), np.ascontiguousarray(lo)


def kernel(hidden_states, cos, sin, wq, bq, wk, wv, bv, wo, bo):
    from concourse.bass_utils import run_bass_kernel_spmd

    def _bf16(x):
        return np.ascontiguousarray(np.asarray(x).astype(ml_dtypes.bfloat16))

    def _f32(x):
        return np.ascontiguousarray(np.asarray(x, dtype=np.float32))

    hsT = np.asarray(hidden_states, dtype=np.float32).reshape(B * T, H).T
    hs8, hsr = _split8(np.ascontiguousarray(hsT))
    cos_t = np.asarray(cos, np.float32).T            # [64, 1024]
    sin_t = np.asarray(sin, np.float32).T
    # sign table indexed by SOURCE row (verifier requires same partitions for
    # both TT inputs): rows 0:32 = +sin[32:64], rows 32:64 = -sin[0:32]
    sin_n = np.concatenate([sin_t[32:64], -sin_t[0:32]], axis=0)
    shared = {
        "cosT2": _bf16(np.concatenate([cos_t, cos_t], axis=0)),
        "sinT2n": _bf16(np.concatenate([sin_n, sin_n], axis=0)),
        "wo": _bf16(wo),
        "bq": _f32(bq), "bv": _bf16(bv), "bo": _bf16(bo),
    }
    for wn, w in (("wq", wq), ("wk", wk), ("wv", wv)):
        hi, lo = _split8(np.asarray(w, dtype=np.float32) * WSC)
        shared[wn + "8"] = hi
        shared[wn + "r"] = lo
    if not _NC_CACHE:
        _NC_CACHE.append(build())
    nc = _NC_CACHE[0]

    in_maps = []
    for c in range(NCORES):
        m = dict(shared)
        m["hs8"] = np.ascontiguousarray(hs8[:, c * TOK:(c + 1) * TOK])
        m["hsr"] = np.ascontiguousarray(hsr[:, c * TOK:(c + 1) * TOK])
        in_maps.append(m)

    try:
        res = run_bass_kernel_spmd(nc, in_maps, core_ids=list(range(NCORES)))
    except Exception:
        # transient NRT device errors (e.g. NRT_EXEC_UNIT_UNRECOVERABLE) have
        # been observed on this fabric; one retry usually succeeds
        time.sleep(2.0)
        res = run_bass_kernel_spmd(nc, in_maps, core_ids=list(range(NCORES)))
    _LAST_RESULT.clear()
    _LAST_RESULT.append(res)
    out = np.concatenate(
        [r["out"].reshape(BPC, T, H) for r in res.results], axis=0)
    return out


# revision 60
# speedup vs baseline: 1.0246x; 1.0246x over previous
"""Dinov3 ViT attention kernel for Trainium2 (8 NeuronCores, data-parallel over batch).

Per core: 2 batch items. hidden_states [2*1029, 1024] in, out [2*1029, 1024] f32.

v2: QKV projections run as fp8e4m3 DoubleRow matmuls with hi+lo error
compensation (host splits X and the QKV weights into e4m3 hi/lo pairs;
weights pre-scaled by 32 so the lo half stays in e4m3 normal range, undone
in the PSUM-evacuation op). Each 128-feature k-tile pair contracts 256 deep
at 0.5 cycles/row, cutting projection PE time 25% at ~bf16 accuracy.
PSUM evacuation (bias/scale) moves to the Pool engine to unload DVE.

Per item pipeline (PE-dense, interleaved with ACT-bound attention):
  X-prep (strided fp8 hi/lo DMAs) ->
  V-proj chunks 0,1 (heads 0..7) ->
  for mo in 0..7:  # one 128-feature tile = head pair (2mo, 2mo+1)
      Q-proj(mo) + bias, K-proj(mo), RoPE(mo) on DVE,
      attention for heads 2mo, 2mo+1:
        S^T per key-tile (K=64 matmul), exp on ScalarE (scale=1/8, no max:
        |scores| < ~7), AV matmul with ones-augmented V (row 64 = softmax sums),
        5-query tail batched into one [128,45] PSUM bank + single exp per head,
        normalize via DVE reciprocal + gpsimd partition_broadcast.
      (V-proj chunk 2 at mo=2, chunk 3 at mo=3)
  output projection Y = (AttnOut^T)^T Wo + bo -> DMA f32.
"""
import sys
import time

sys.path.insert(0, "/opt/trn_rl_repo")

import ml_dtypes
import numpy as np

import concourse.bacc as bacc
import concourse.mybir as mybir
import concourse.tile as tile

f32 = mybir.dt.float32
bf16 = mybir.dt.bfloat16
fp8 = mybir.dt.float8e4
FP = mybir.ActivationFunctionType
ADD = mybir.AluOpType.add
MUL = mybir.AluOpType.mult
SUB = mybir.AluOpType.subtract
DR = mybir.MatmulPerfMode.DoubleRow

H = 1024
NH = 16
HD = 64
T = 1029
NPREF = 5
PATCH = 1024
B = 16
NCORES = 8
BPC = B // NCORES          # batch items per core
KO = H // 128              # 8 feature k-tiles
TOK = BPC * T              # tokens per core (2058)
SCALE = 1.0 / float(np.sqrt(HD))
WSC = 32.0                 # host pre-scale on fp8-split qkv weights
IWSC = float(1.0 / WSC)

TPAD = 1040                # dual-fp8 ldweights needs 16B-multiple pair stride
TOK_TILES = [(i * 128, min(128, T - i * 128)) for i in range((T + 127) // 128)]
NJT = len(TOK_TILES)
QCHUNKS = [(0, 512), (512, 512)]
QTAIL = (1024, T - 1024)               # 5 queries -> batched-exp path
# DoubleRow moving chunks: rhs free = 2*qw <= 512 -> qw <= 256
DRQ_CHUNKS = [(0, 256), (256, 256), (512, 256), (768, 256), (1024, 5)]
VN_CHUNKS = [(0, 256), (256, 256), (512, 256), (768, 256)]  # V out features


def build():
    nc = bacc.Bacc(None, target_bir_lowering=False)
    hs8_d = nc.dram_tensor("hs8", [H, TOK], fp8, kind="ExternalInput")
    hsr_d = nc.dram_tensor("hsr", [H, TOK], fp8, kind="ExternalInput")
    # host-prepared rope tables: cosT2[d, t] = cos[t, d%64] duplicated;
    # sinT2sw = swapped/sign-flipped sin layout (see emit_rope_t)
    cosT2_d = nc.dram_tensor("cosT2", [128, PATCH], bf16, kind="ExternalInput")
    sinT2n_d = nc.dram_tensor("sinT2n", [128, PATCH], bf16,
                              kind="ExternalInput")
    w_d = {}
    for wn in ("wq", "wk", "wv"):
        w_d[wn + "8"] = nc.dram_tensor(wn + "8", [H, H], fp8,
                                       kind="ExternalInput")
        w_d[wn + "r"] = nc.dram_tensor(wn + "r", [H, H], fp8,
                                       kind="ExternalInput")
    w_d["wo"] = nc.dram_tensor("wo", [H, H], bf16, kind="ExternalInput")
    b_d = {"bq": nc.dram_tensor("bq", [H], f32, kind="ExternalInput"),
           "bv": nc.dram_tensor("bv", [H], bf16, kind="ExternalInput"),
           "bo": nc.dram_tensor("bo", [H], bf16, kind="ExternalInput")}
    out_d = nc.dram_tensor("out", [TOK, H], f32, kind="ExternalOutput")

    with tile.TileContext(nc) as tc:
        with (
            tc.tile_pool(name="const", bufs=1) as cpool,
            tc.tile_pool(name="item", bufs=1) as ipool,
            tc.tile_pool(name="ao", bufs=2) as aopool,
            tc.tile_pool(name="work", bufs=3) as wpool,
            tc.tile_pool(name="rope", bufs=1) as rpool,
            tc.tile_pool(name="stack", bufs=2) as spool,
            tc.tile_pool(name="stack1", bufs=1) as spool1,
            tc.tile_pool(name="attn", bufs=3) as apool,
            tc.tile_pool(name="ypool", bufs=2) as ypool,
            tc.tile_pool(name="attn2", bufs=1) as apool2,
            tc.tile_pool(name="ps_s", bufs=2, space="PSUM") as ps_s,
            tc.tile_pool(name="ps_o", bufs=1, space="PSUM") as ps_o,
            tc.tile_pool(name="ps_w", bufs=2, space="PSUM") as ps_w,
        ):
            # --- X-prep: hs pre-split to fp8 hi/lo, feature-major on host ---
            hs8_r = hs8_d.rearrange("(o p) t -> p o t", p=128)
            hsr_r = hsr_d.rearrange("(o p) t -> p o t", p=128)

            def emit_xprep_full(bi, X8, XR):
                nc.sync.dma_start(X8[:, :, 0:T],
                                  hs8_r[:, :, bi * T: bi * T + T])
                nc.sync.dma_start(XR[:, :, 0:T],
                                  hsr_r[:, :, bi * T: bi * T + T])

            X8_0 = ipool.tile([128, KO, TPAD], fp8, tag="X8", name="X8_0")
            XR_0 = ipool.tile([128, KO, TPAD], fp8, tag="XR", name="XR_0")
            emit_xprep_full(0, X8_0, XR_0)

            # --- rope tables straight from host (ACT hwdge queue: keep the
            # SP queue free for the X stream) ---
            cosT2 = cpool.tile([128, PATCH], bf16)
            sinT2n = cpool.tile([128, PATCH], bf16)
            nc.scalar.dma_start(cosT2[:], cosT2_d[:])
            nc.scalar.dma_start(sinT2n[:], sinT2n_d[:])

            # --- biases, weights (single strided DMA each, ACT queue,
            # ordered by first use: V-proj then Q/K-proj then out-proj) ---
            bq_sb = cpool.tile([128, KO], f32)
            nc.scalar.dma_start(bq_sb[:],
                                b_d["bq"].rearrange("(o p) -> p o", p=128))
            bv_bc = cpool.tile([128, H], bf16)
            nc.scalar.dma_start(bv_bc[:],
                                b_d["bv"][None, :].to_broadcast((128, H)))
            bo_row = cpool.tile([1, H], bf16)
            nc.scalar.dma_start(bo_row[:], b_d["bo"][None, :])
            ones_row = cpool.tile([1, 128], bf16)
            nc.vector.memset(ones_row[:], 1.0)

            wb = {}
            for wn in ("wq8", "wqr", "wk8", "wkr", "wv8", "wvr"):
                wb[wn] = cpool.tile([128, KO, H], fp8, tag=f"wb_{wn}",
                                    name=f"wb_{wn}")
            wb["wo"] = cpool.tile([128, KO, H], bf16, tag="wb_wo", name="wb_wo")
            for wn in ("wv8", "wvr", "wq8", "wqr", "wk8", "wkr", "wo"):
                nc.scalar.dma_start(
                    wb[wn][:], w_d[wn].rearrange("(o p) n -> p o n", p=128))

            # ---------------- per batch item ----------------
            def make_item(bi, X8, XR):
                tok0 = bi * T
                QT = ipool.tile([128, KO, T], bf16, tag="QT", name=f"QT_{bi}")
                KT = ipool.tile([128, KO, T], bf16, tag="KT", name=f"KT_{bi}")
                Vst = ipool.tile([128, NJT, NH, HD + 1], bf16, tag="Vst",
                                 name=f"Vst_{bi}")
                AOT = aopool.tile([128, KO, T], bf16, tag="AOT", name=f"AOT_{bi}")

                def emit_vinit():
                    nc.vector.memset(Vst[:, :, :, HD:HD + 1], 1.0)

                def emit_vproj_t(ci, ti):
                    n0, nw = VN_CHUNKS[ci]
                    t0, tw = TOK_TILES[ti]
                    pm = ps_w.tile([128, 512], f32, tag="ps_w",
                                   name=f"pmv_{bi}_{ci}_{ti}")
                    # 3-term compensated DR: X8*W8, X8*Wr, Xr*W8
                    steps = ([(X8, "wv8")] * 4 + [(X8, "wvr")] * 4
                             + [(XR, "wv8")] * 4)
                    for si, (xt, wn) in enumerate(steps):
                        j = 2 * (si % 4)
                        nc.tensor.matmul(
                            pm[:tw, :nw],
                            xt[:, j:j + 2, t0:t0 + tw],
                            wb[wn][:, j:j + 2, n0:n0 + nw],
                            start=(si == 0), stop=(si == len(steps) - 1),
                            perf_mode=DR)
                    nc.vector.scalar_tensor_tensor(
                        Vst[:tw, ti, n0 // HD:(n0 + nw) // HD, 0:HD],
                        pm[:tw, :nw], IWSC, bv_bc[:tw, n0:n0 + nw],
                        op0=MUL, op1=ADD)

                def emit_qkproj_g(mo, which, ci):
                    dst, w8n, wrn, bias = (
                        (QT, "wq8", "wqr", True), (KT, "wk8", "wkr", False)
                    )[which]
                    q0, qw = DRQ_CHUNKS[ci]
                    pm = ps_w.tile([128, 512], f32, tag="ps_w",
                                   name=f"pm_{bi}_{w8n}_{mo}_{q0}")
                    steps = ([(X8, w8n)] * 4 + [(XR, w8n)] * 4
                             + [(X8, wrn)] * 4)
                    for si, (xt, wn) in enumerate(steps):
                        j = 2 * (si % 4)
                        nc.tensor.matmul(
                            pm[:, :qw],
                            wb[wn][:, j:j + 2, mo * 128:(mo + 1) * 128],
                            xt[:, j:j + 2, q0:q0 + qw],
                            start=(si == 0), stop=(si == len(steps) - 1),
                            perf_mode=DR)
                    if bias:
                        nc.vector.scalar_tensor_tensor(
                            dst[:, mo, q0:q0 + qw], pm[:, :qw], IWSC,
                            bq_sb[:, mo:mo + 1].to_broadcast((128, qw)),
                            op0=MUL, op1=ADD)
                    else:
                        nc.vector.tensor_scalar_mul(
                            dst[:, mo, q0:q0 + qw], pm[:, :qw], IWSC)

                stacks = {}

                def emit_stack_q(mo):
                    # fp8 hi/lo operand stacks for DoubleRow scores:
                    #   SQ_h = [[q8;q8],[qr;qr]]  (rhs slices)
                    #   SK_h = [k8;kr]            (lhsT, used for both slices)
                    # S = (k8+kr)^T (q8+qr): full hi/lo product, ~bf16 accuracy
                    q8f = spool1.tile([128, TPAD], fp8, tag="q8f")
                    nc.scalar.copy(q8f[:, 0:T], QT[:, mo, :])
                    qrf = spool1.tile([128, TPAD], fp8, tag="qrf")
                    nc.vector.tensor_tensor(qrf[:, 0:T], QT[:, mo, :],
                                            q8f[:, 0:T], SUB)
                    sqs = []
                    for half in range(2):
                        ph = half * 64
                        sq = spool.tile([128, 2, TPAD], fp8, tag=f"sq{half}")
                        for dst_ph in (0, 64):
                            nc.sync.dma_start(sq[dst_ph:dst_ph + 64, 0, 0:T],
                                              q8f[ph:ph + 64, 0:T])
                            nc.sync.dma_start(sq[dst_ph:dst_ph + 64, 1, 0:T],
                                              qrf[ph:ph + 64, 0:T])
                        sqs.append(sq)
                    stacks.setdefault(mo, [[None, None], [None, None]])
                    stacks[mo][0][0] = sqs[0]
                    stacks[mo][1][0] = sqs[1]

                def emit_stack_k(mo):
                    k8f = spool1.tile([128, TPAD], fp8, tag="k8f")
                    nc.scalar.copy(k8f[:, 0:T], KT[:, mo, :])
                    krf = spool1.tile([128, TPAD], fp8, tag="krf")
                    nc.vector.tensor_tensor(krf[:, 0:T], KT[:, mo, :],
                                            k8f[:, 0:T], SUB)
                    for half in range(2):
                        ph = half * 64
                        sk = spool.tile([128, TPAD], fp8, tag=f"sk{half}")
                        nc.scalar.dma_start(sk[0:64, 0:T], k8f[ph:ph + 64, 0:T])
                        nc.scalar.dma_start(sk[64:128, 0:T],
                                            krf[ph:ph + 64, 0:T])
                        stacks[mo][half][1] = sk

                def emit_rope_t(mo, which):
                    tgt = (QT, KT)[which]
                    src = tgt[:, mo, NPREF:T]
                    t1 = rpool.tile([128, PATCH], bf16, tag="rope1")
                    nc.vector.tensor_tensor(t1[:], src, cosT2[:], MUL)
                    t2 = rpool.tile([128, PATCH], bf16, tag="rope2")
                    for (o, sp) in ((0, 32), (32, 0), (64, 96), (96, 64)):
                        nc.vector.tensor_tensor(
                            t2[o:o + 32, :], tgt[sp:sp + 32, mo, NPREF:T],
                            sinT2n[sp:sp + 32, :], MUL)
                    nc.vector.tensor_tensor(src, t1[:], t2[:], ADD)

                def emit_attn(h, pump=None):
                    ph = (h % 2) * 64
                    kq = h // 2
                    po = ps_o.tile([128, 1024], f32, tag="ps_o",
                                   name=f"po_{bi}_{h}")

                    sq, sk = stacks[kq][h % 2]

                    def s_mm(ji):
                        j0, jw = TOK_TILES[ji]
                        pss = ps_s.tile([128, 1024], f32, tag="ps_s")
                        for q0 in range(0, 1024, 256):
                            nc.tensor.matmul(
                                pss[:jw, q0:q0 + 256],
                                sk[:, None, j0:j0 + jw].to_broadcast(
                                    (128, 2, jw)),
                                sq[:, :, q0:q0 + 256],
                                start=True, stop=True, perf_mode=DR)
                        return pss

                    # software pipeline: S(ji+1) and pump work sit between
                    # exp(ji) and AV(ji) in the in-order PE stream, so the PE
                    # computes next-tile scores while ACT runs exp(ji)
                    pss = s_mm(0)
                    for ji, (j0, jw) in enumerate(TOK_TILES):
                        first, last = ji == 0, ji == NJT - 1
                        es = apool.tile([128, 1024], bf16, tag="expS")
                        nc.scalar.activation(es[:jw, :], pss[:jw, :],
                                             FP.Exp, scale=SCALE)
                        if not last:
                            pss = s_mm(ji + 1)
                        if pump is not None:
                            pump()
                        for q0, qw in QCHUNKS:
                            nc.tensor.matmul(
                                po[:HD + 1, q0:q0 + qw],
                                Vst[:jw, ji, h, :],
                                es[:jw, q0:q0 + qw],
                                start=first, stop=last)
                    for qi, (q0, qw) in enumerate(QCHUNKS):
                        rc = apool2.tile([1, 512], f32, tag="recip")
                        nc.vector.reciprocal(rc[0:1, :qw], po[64:65, q0:q0 + qw])
                        rb = apool2.tile([64, 512], f32, tag="recipB")
                        nc.gpsimd.partition_broadcast(rb[:, :qw], rc[0:1, :qw])
                        nc.vector.tensor_tensor(
                            AOT[ph:ph + 64, kq, q0:q0 + qw],
                            po[0:64, q0:q0 + qw], rb[:, :qw], MUL)

                def emit_tail():
                    # 5-query tail for all 16 heads, batched: S packed into one
                    # ps_s slot (heads 0..10 bank A, 11..15 bank B), two exps,
                    # AV accumulated per head into one ps_o slot.
                    qt0, qtw = QTAIL
                    pst = ps_s.tile([128, 1024], f32, tag="ps_s",
                                    name=f"pst_{bi}")
                    nc.vector.memset(pst[:], 0.0)

                    def tcol(h):
                        return (h * qtw * NJT if h <= 10
                                else 512 + (h - 11) * qtw * NJT)

                    for h in range(NH):
                        ph = (h % 2) * 64
                        kq = h // 2
                        for ji, (j0, jw) in enumerate(TOK_TILES):
                            nc.tensor.matmul(
                                pst[:jw,
                                    tcol(h) + ji * qtw: tcol(h) + (ji + 1) * qtw],
                                KT[ph:ph + 64, kq, j0:j0 + jw],
                                QT[ph:ph + 64, kq, qt0:qt0 + qtw],
                                start=True, stop=True)
                    est = apool.tile([128, 1024], bf16, tag="expS",
                                     name=f"est_{bi}")
                    nc.scalar.activation(est[:, 0:495], pst[:, 0:495],
                                         FP.Exp, scale=SCALE)
                    nc.scalar.activation(est[:, 512:737], pst[:, 512:737],
                                         FP.Exp, scale=SCALE)
                    pot = ps_o.tile([128, 1024], f32, tag="ps_o",
                                    name=f"pot_{bi}")
                    for h in range(NH):
                        for ji, (j0, jw) in enumerate(TOK_TILES):
                            nc.tensor.matmul(
                                pot[:HD + 1, h * qtw:(h + 1) * qtw],
                                Vst[:jw, ji, h, :],
                                est[0:jw,
                                    tcol(h) + ji * qtw: tcol(h) + (ji + 1) * qtw],
                                start=(ji == 0), stop=(ji == NJT - 1))
                    rc = apool2.tile([1, 512], f32, tag="recip")
                    nc.vector.reciprocal(rc[0:1, :NH * qtw],
                                         pot[64:65, :NH * qtw])
                    rb = apool2.tile([64, 512], f32, tag="recipB")
                    nc.gpsimd.partition_broadcast(rb[:, :NH * qtw],
                                                  rc[0:1, :NH * qtw])
                    for h in range(NH):
                        nc.vector.tensor_tensor(
                            AOT[(h % 2) * 64:(h % 2) * 64 + 64, h // 2,
                                qt0:qt0 + qtw],
                            pot[0:64, h * qtw:(h + 1) * qtw],
                            rb[:, h * qtw:(h + 1) * qtw], MUL)

                def emit_outproj_g(ti, nci):
                    t0, tw = TOK_TILES[ti]
                    n0, nw = nci * 512, 512
                    pm = ps_w.tile([128, 512], f32, tag="ps_w",
                                   name=f"pmo_{bi}_{ti}_{n0}")
                    for ko in range(KO):
                        nc.tensor.matmul(
                            pm[:tw, :nw],
                            AOT[:, ko, t0:t0 + tw],
                            wb["wo"][:, ko, n0:n0 + nw],
                            start=(ko == 0), stop=False)
                    # K=1 ones-row matmul adds the bias; evac on ACT (the
                    # drain phase is DVE-bound, ACT idles there)
                    nc.tensor.matmul(
                        pm[:tw, :nw], ones_row[0:1, 0:tw],
                        bo_row[0:1, n0:n0 + nw], start=False, stop=True)
                    y = ypool.tile([128, 512], f32, tag="y")
                    nc.scalar.copy(y[:tw, :nw], pm[:tw, :nw])
                    nc.sync.dma_start(
                        out_d[tok0 + t0: tok0 + t0 + tw, n0:n0 + nw],
                        y[:tw, :nw])

                def emit_outproj(skip=()):
                    for ti in range(NJT):
                        for nci in range(2):
                            if (ti, nci) not in skip:
                                emit_outproj_g(ti, nci)

                def emit_blocks(extra=None):
                    # two queues: crit (next mo's q/k proj + rope — MUST be
                    # emitted within this mo's 18 pump slots: program order is
                    # semantics) and bulk (V chunks, deferred out-proj, xprep
                    # — emitted whenever slots are free, may spill)
                    crit = []
                    bulk = []

                    def pump():
                        if crit:
                            crit.pop(0)()
                        elif bulk:
                            bulk.pop(0)()

                    for mo in range(KO):
                        if mo < KO - 1:
                            crit.extend(
                                (lambda mo=mo, which=which, ci=ci:
                                 emit_qkproj_g(mo + 1, which, ci))
                                for which in range(2)
                                for ci in range(len(DRQ_CHUNKS)))
                            crit.append(lambda mo=mo: emit_rope_t(mo + 1, 0))
                            crit.append(lambda mo=mo: emit_stack_q(mo + 1))
                            crit.append(lambda mo=mo: emit_rope_t(mo + 1, 1))
                            crit.append(lambda mo=mo: emit_stack_k(mo + 1))
                        if mo == 0:
                            bulk.extend(
                                (lambda ti=ti: emit_vproj_t(2, ti))
                                for ti in range(NJT))
                        if mo == 1:
                            bulk.extend(
                                (lambda ti=ti: emit_vproj_t(3, ti))
                                for ti in range(NJT))
                        if extra and mo in extra:
                            bulk.extend(extra[mo])
                        emit_attn(2 * mo, pump)
                        emit_attn(2 * mo + 1, pump)
                    while crit or bulk:
                        pump()

                def emit_head():
                    emit_vinit()
                    for ci in range(2):
                        for ti in range(NJT):
                            emit_vproj_t(ci, ti)
                    for which in range(2):
                        for ci in range(len(DRQ_CHUNKS)):
                            emit_qkproj_g(0, which, ci)
                    emit_rope_t(0, 0)
                    emit_stack_q(0)
                    emit_rope_t(0, 1)
                    emit_stack_k(0)

                return {
                    "head": emit_head, "blocks": emit_blocks,
                    "tail": emit_tail, "outproj": emit_outproj,
                    "outproj_g": emit_outproj_g,
                }

            it0 = make_item(0, X8_0, XR_0)
            it0["head"]()
            X8_1 = ipool.tile([128, KO, TPAD], fp8, tag="X8", name="X8_1")
            XR_1 = ipool.tile([128, KO, TPAD], fp8, tag="XR", name="XR_1")
            it0["blocks"](extra={6: [lambda: emit_xprep_full(1, X8_1, XR_1)]})
            it0["tail"]()
            it1 = make_item(1, X8_1, XR_1)
            it1["head"]()              # runs during item0 out-proj
            # spread most of item0's out-proj through item1's attention as
            # extra pump work (2 chunks per mo)
            defer = [(ti, nci) for ti in range(1, NJT) for nci in range(2)]
            it0["outproj"](skip=defer)
            dthunks = [(lambda ti=ti, nci=nci: it0["outproj_g"](ti, nci))
                       for (ti, nci) in defer]
            extra = {mo: dthunks[mo * 2: mo * 2 + 2] for mo in range(7)}
            extra[7] = dthunks[14:]
            it1["blocks"](extra=extra)
            it1["tail"]()
            it1["outproj"]()

    nc.compile()
    return nc


_NC_CACHE = []
_LAST_RESULT = []


def _e4m3(x):
    return np.ascontiguousarray(x.astype(ml_dtypes.float8_e4m3))


def _split8(x):
    """x (f32) -> (hi, lo) e4m3 pair with hi + lo ~ x."""
    hi = x.astype(ml_dtypes.float8_e4m3)
    lo = (x - hi.astype(np.float32)).astype(ml_dtypes.float8_e4m3)
    return np.ascontiguousarray(hi), np.ascontiguousarray(lo)


def kernel(hidden_states, cos, sin, wq, bq, wk, wv, bv, wo, bo):
    from concourse.bass_utils import run_bass_kernel_spmd

    def _bf16(x):
        return np.ascontiguousarray(np.asarray(x).astype(ml_dtypes.bfloat16))

    def _f32(x):
        return np.ascontiguousarray(np.asarray(x, dtype=np.float32))

    hsT = np.asarray(hidden_states, dtype=np.float32).reshape(B * T, H).T
    hs8, hsr = _split8(np.ascontiguousarray(hsT))
    cos_t = np.asarray(cos, np.float32).T            # [64, 1024]
    sin_t = np.asarray(sin, np.float32).T
    # sign table indexed by SOURCE row (verifier requires same partitions for
    # both TT inputs): rows 0:32 = +sin[32:64], rows 32:64 = -sin[0:32]
    sin_n = np.concatenate([sin_t[32:64], -sin_t[0:32]], axis=0)
    shared = {
        "cosT2": _bf16(np.concatenate([cos_t, cos_t], axis=0)),
        "sinT2n": _bf16(np.concatenate([sin_n, sin_n], axis=0)),
        "wo": _bf16(wo),
        "bq": _f32(bq), "bv": _bf16(bv), "bo": _bf16(bo),
    }
    for wn, w in (("wq", wq), ("wk", wk), ("wv", wv)):
        hi, lo = _split8(np.asarray(w, dtype=np.float32) * WSC)
        shared[wn + "8"] = hi
        shared[wn + "r"] = lo
    if not _NC_CACHE:
        _NC_CACHE.append(build())
    nc = _NC_CACHE[0]

    in_maps = []
    for c in range(NCORES):
        m = dict(shared)
        m["hs8"] = np.ascontiguousarray(hs8[:, c * TOK:(c + 1) * TOK])
        m["hsr"] = np.ascontiguousarray(hsr[:, c * TOK:(c + 1) * TOK])
        in_maps.append(m)

    try:
        res = run_bass_kernel_spmd(nc, in_maps, core_ids=list(range(NCORES)))
    except Exception:
        # transient NRT device errors (e.g. NRT_EXEC_UNIT_UNRECOVERABLE) have
        # been observed on this fabric; one retry usually succeeds
        time.sleep(2.0)
        res = run_bass_kernel_spmd(nc, in_maps, core_ids=list(range(NCORES)))
    _LAST_RESULT.clear()
    _LAST_RESULT.append(res)
    out = np.concatenate(
        [r["out"].reshape(BPC, T, H) for r in res.results], axis=0)
    return out


# revision 61
# speedup vs baseline: 1.0467x; 1.0216x over previous
"""Dinov3 ViT attention kernel for Trainium2 (8 NeuronCores, data-parallel over batch).

Per core: 2 batch items. hidden_states [2*1029, 1024] in, out [2*1029, 1024] f32.

v2: QKV projections run as fp8e4m3 DoubleRow matmuls with hi+lo error
compensation (host splits X and the QKV weights into e4m3 hi/lo pairs;
weights pre-scaled by 32 so the lo half stays in e4m3 normal range, undone
in the PSUM-evacuation op). Each 128-feature k-tile pair contracts 256 deep
at 0.5 cycles/row, cutting projection PE time 25% at ~bf16 accuracy.
PSUM evacuation (bias/scale) moves to the Pool engine to unload DVE.

Per item pipeline (PE-dense, interleaved with ACT-bound attention):
  X-prep (strided fp8 hi/lo DMAs) ->
  V-proj chunks 0,1 (heads 0..7) ->
  for mo in 0..7:  # one 128-feature tile = head pair (2mo, 2mo+1)
      Q-proj(mo) + bias, K-proj(mo), RoPE(mo) on DVE,
      attention for heads 2mo, 2mo+1:
        S^T per key-tile (K=64 matmul), exp on ScalarE (scale=1/8, no max:
        |scores| < ~7), AV matmul with ones-augmented V (row 64 = softmax sums),
        5-query tail batched into one [128,45] PSUM bank + single exp per head,
        normalize via DVE reciprocal + gpsimd partition_broadcast.
      (V-proj chunk 2 at mo=2, chunk 3 at mo=3)
  output projection Y = (AttnOut^T)^T Wo + bo -> DMA f32.
"""
import sys
import time

sys.path.insert(0, "/opt/trn_rl_repo")

import ml_dtypes
import numpy as np

import concourse.bacc as bacc
import concourse.mybir as mybir
import concourse.tile as tile

f32 = mybir.dt.float32
bf16 = mybir.dt.bfloat16
fp8 = mybir.dt.float8e4
FP = mybir.ActivationFunctionType
ADD = mybir.AluOpType.add
MUL = mybir.AluOpType.mult
SUB = mybir.AluOpType.subtract
DR = mybir.MatmulPerfMode.DoubleRow

H = 1024
NH = 16
HD = 64
T = 1029
NPREF = 5
PATCH = 1024
B = 16
NCORES = 8
BPC = B // NCORES          # batch items per core
KO = H // 128              # 8 feature k-tiles
TOK = BPC * T              # tokens per core (2058)
SCALE = 1.0 / float(np.sqrt(HD))
WSC = 32.0                 # host pre-scale on fp8-split qkv weights
IWSC = float(1.0 / WSC)

TPAD = 1040                # dual-fp8 ldweights needs 16B-multiple pair stride
TOK_TILES = [(i * 128, min(128, T - i * 128)) for i in range((T + 127) // 128)]
NJT = len(TOK_TILES)
QCHUNKS = [(0, 512), (512, 512)]
QTAIL = (1024, T - 1024)               # 5 queries -> batched-exp path
# DoubleRow moving chunks: rhs free = 2*qw <= 512 -> qw <= 256
DRQ_CHUNKS = [(0, 256), (256, 256), (512, 256), (768, 256), (1024, 5)]
VN_CHUNKS = [(0, 256), (256, 256), (512, 256), (768, 256)]  # V out features


def build():
    nc = bacc.Bacc(None, target_bir_lowering=False)
    hs8_d = nc.dram_tensor("hs8", [H, TOK], fp8, kind="ExternalInput")
    hsr_d = nc.dram_tensor("hsr", [H, TOK], fp8, kind="ExternalInput")
    # host-prepared rope tables: cosT2[d, t] = cos[t, d%64] duplicated;
    # sinT2sw = swapped/sign-flipped sin layout (see emit_rope_t)
    cosT2_d = nc.dram_tensor("cosT2", [128, PATCH], bf16, kind="ExternalInput")
    sinT2n_d = nc.dram_tensor("sinT2n", [128, PATCH], bf16,
                              kind="ExternalInput")
    w_d = {}
    for wn in ("wq", "wk", "wv"):
        w_d[wn + "8"] = nc.dram_tensor(wn + "8", [H, H], fp8,
                                       kind="ExternalInput")
        w_d[wn + "r"] = nc.dram_tensor(wn + "r", [H, H], fp8,
                                       kind="ExternalInput")
    w_d["wo"] = nc.dram_tensor("wo", [H, H], bf16, kind="ExternalInput")
    b_d = {"bq": nc.dram_tensor("bq", [H], f32, kind="ExternalInput"),
           "bv": nc.dram_tensor("bv", [H], bf16, kind="ExternalInput"),
           "bo": nc.dram_tensor("bo", [H], bf16, kind="ExternalInput")}
    out_d = nc.dram_tensor("out", [TOK, H], f32, kind="ExternalOutput")

    with tile.TileContext(nc) as tc:
        with (
            tc.tile_pool(name="const", bufs=1) as cpool,
            tc.tile_pool(name="item", bufs=1) as ipool,
            tc.tile_pool(name="ao", bufs=2) as aopool,
            tc.tile_pool(name="work", bufs=3) as wpool,
            tc.tile_pool(name="rope", bufs=1) as rpool,
            tc.tile_pool(name="stack", bufs=2) as spool,
            tc.tile_pool(name="stack1", bufs=1) as spool1,
            tc.tile_pool(name="attn", bufs=3) as apool,
            tc.tile_pool(name="ypool", bufs=2) as ypool,
            tc.tile_pool(name="attn2", bufs=1) as apool2,
            tc.tile_pool(name="ps_s", bufs=2, space="PSUM") as ps_s,
            tc.tile_pool(name="ps_o", bufs=1, space="PSUM") as ps_o,
            tc.tile_pool(name="ps_w", bufs=2, space="PSUM") as ps_w,
        ):
            # --- X-prep: hs pre-split to fp8 hi/lo, feature-major on host ---
            hs8_r = hs8_d.rearrange("(o p) t -> p o t", p=128)
            hsr_r = hsr_d.rearrange("(o p) t -> p o t", p=128)

            def emit_xprep_full(bi, X8, XR):
                nc.sync.dma_start(X8[:, :, 0:T],
                                  hs8_r[:, :, bi * T: bi * T + T])
                nc.sync.dma_start(XR[:, :, 0:T],
                                  hsr_r[:, :, bi * T: bi * T + T])

            X8_0 = ipool.tile([128, KO, TPAD], fp8, tag="X8", name="X8_0")
            XR_0 = ipool.tile([128, KO, TPAD], fp8, tag="XR", name="XR_0")
            emit_xprep_full(0, X8_0, XR_0)

            # --- rope tables straight from host (ACT hwdge queue: keep the
            # SP queue free for the X stream) ---
            cosT2 = cpool.tile([128, PATCH], bf16)
            sinT2n = cpool.tile([128, PATCH], bf16)
            nc.scalar.dma_start(cosT2[:], cosT2_d[:])
            nc.scalar.dma_start(sinT2n[:], sinT2n_d[:])

            # --- biases, weights (single strided DMA each, ACT queue,
            # ordered by first use: V-proj then Q/K-proj then out-proj) ---
            bq_sb = cpool.tile([128, KO], f32)
            nc.scalar.dma_start(bq_sb[:],
                                b_d["bq"].rearrange("(o p) -> p o", p=128))
            bv_bc = cpool.tile([128, H], bf16)
            nc.scalar.dma_start(bv_bc[:],
                                b_d["bv"][None, :].to_broadcast((128, H)))
            bo_row = cpool.tile([1, H], bf16)
            nc.scalar.dma_start(bo_row[:], b_d["bo"][None, :])
            ones_row = cpool.tile([1, 128], bf16)
            nc.vector.memset(ones_row[:], 1.0)

            wb = {}
            for wn in ("wq8", "wqr", "wk8", "wkr", "wv8", "wvr"):
                wb[wn] = cpool.tile([128, KO, H], fp8, tag=f"wb_{wn}",
                                    name=f"wb_{wn}")
            wb["wo"] = cpool.tile([128, KO, H], bf16, tag="wb_wo", name="wb_wo")
            for wn in ("wv8", "wvr", "wq8", "wqr", "wk8", "wkr", "wo"):
                nc.scalar.dma_start(
                    wb[wn][:], w_d[wn].rearrange("(o p) n -> p o n", p=128))

            # ---------------- per batch item ----------------
            def make_item(bi, X8, XR):
                tok0 = bi * T
                QT = ipool.tile([128, KO, T], bf16, tag="QT", name=f"QT_{bi}")
                KT = ipool.tile([128, KO, T], bf16, tag="KT", name=f"KT_{bi}")
                Vst = ipool.tile([128, NJT, NH, HD + 1], bf16, tag="Vst",
                                 name=f"Vst_{bi}")
                AOT = aopool.tile([128, KO, T], bf16, tag="AOT", name=f"AOT_{bi}")

                def emit_vinit():
                    nc.vector.memset(Vst[:, :, :, HD:HD + 1], 1.0)

                def emit_vproj_t(ci, ti):
                    n0, nw = VN_CHUNKS[ci]
                    t0, tw = TOK_TILES[ti]
                    pm = ps_w.tile([128, 512], f32, tag="ps_w",
                                   name=f"pmv_{bi}_{ci}_{ti}")
                    # 3-term compensated DR: X8*W8, X8*Wr, Xr*W8
                    steps = ([(X8, "wv8")] * 4 + [(X8, "wvr")] * 4
                             + [(XR, "wv8")] * 4)
                    for si, (xt, wn) in enumerate(steps):
                        j = 2 * (si % 4)
                        nc.tensor.matmul(
                            pm[:tw, :nw],
                            xt[:, j:j + 2, t0:t0 + tw],
                            wb[wn][:, j:j + 2, n0:n0 + nw],
                            start=(si == 0), stop=(si == len(steps) - 1),
                            perf_mode=DR)
                    nc.vector.scalar_tensor_tensor(
                        Vst[:tw, ti, n0 // HD:(n0 + nw) // HD, 0:HD],
                        pm[:tw, :nw], IWSC, bv_bc[:tw, n0:n0 + nw],
                        op0=MUL, op1=ADD)

                def emit_qkproj_g(mo, which, ci):
                    dst, w8n, wrn, bias = (
                        (QT, "wq8", "wqr", True), (KT, "wk8", "wkr", False)
                    )[which]
                    q0, qw = DRQ_CHUNKS[ci]
                    pm = ps_w.tile([128, 512], f32, tag="ps_w",
                                   name=f"pm_{bi}_{w8n}_{mo}_{q0}")
                    steps = ([(X8, w8n)] * 4 + [(XR, w8n)] * 4
                             + [(X8, wrn)] * 4)
                    for si, (xt, wn) in enumerate(steps):
                        j = 2 * (si % 4)
                        nc.tensor.matmul(
                            pm[:, :qw],
                            wb[wn][:, j:j + 2, mo * 128:(mo + 1) * 128],
                            xt[:, j:j + 2, q0:q0 + qw],
                            start=(si == 0), stop=(si == len(steps) - 1),
                            perf_mode=DR)
                    if bias:
                        nc.vector.scalar_tensor_tensor(
                            dst[:, mo, q0:q0 + qw], pm[:, :qw], IWSC,
                            bq_sb[:, mo:mo + 1].to_broadcast((128, qw)),
                            op0=MUL, op1=ADD)
                    else:
                        nc.vector.tensor_scalar_mul(
                            dst[:, mo, q0:q0 + qw], pm[:, :qw], IWSC)

                stacks = {}

                def emit_stack(mo):
                    # fp8 hi/lo operand stacks for DoubleRow scores:
                    #   SQ_h = [[q8;q8],[qr;qr]]  (rhs slices)
                    #   SK_h = [k8;kr]            (lhsT, used for both slices)
                    # S = (k8+kr)^T (q8+qr): full hi/lo product, ~bf16 accuracy
                    q8f = spool1.tile([128, TPAD], fp8, tag="q8f")
                    nc.scalar.copy(q8f[:, 0:T], QT[:, mo, :])
                    qrf = spool1.tile([128, TPAD], fp8, tag="qrf")
                    nc.vector.tensor_tensor(qrf[:, 0:T], QT[:, mo, :],
                                            q8f[:, 0:T], SUB)
                    k8f = spool1.tile([128, TPAD], fp8, tag="k8f")
                    nc.scalar.copy(k8f[:, 0:T], KT[:, mo, :])
                    krf = spool1.tile([128, TPAD], fp8, tag="krf")
                    nc.vector.tensor_tensor(krf[:, 0:T], KT[:, mo, :],
                                            k8f[:, 0:T], SUB)
                    tiles = []
                    for half in range(2):
                        ph = half * 64
                        sq = spool.tile([128, 2, TPAD], fp8, tag=f"sq{half}")
                        sk = spool.tile([128, TPAD], fp8, tag=f"sk{half}")
                        for dst_ph in (0, 64):
                            nc.sync.dma_start(sq[dst_ph:dst_ph + 64, 0, 0:T],
                                              q8f[ph:ph + 64, 0:T])
                            nc.sync.dma_start(sq[dst_ph:dst_ph + 64, 1, 0:T],
                                              qrf[ph:ph + 64, 0:T])
                        nc.sync.dma_start(sk[0:64, 0:T], k8f[ph:ph + 64, 0:T])
                        nc.sync.dma_start(sk[64:128, 0:T], krf[ph:ph + 64, 0:T])
                        tiles.append((sq, sk))
                    stacks[mo] = tiles

                def emit_rope_t(mo, which):
                    tgt = (QT, KT)[which]
                    src = tgt[:, mo, NPREF:T]
                    t1 = rpool.tile([128, PATCH], bf16, tag="rope1")
                    nc.vector.tensor_tensor(t1[:], src, cosT2[:], MUL)
                    t2 = rpool.tile([128, PATCH], bf16, tag="rope2")
                    for (o, sp) in ((0, 32), (32, 0), (64, 96), (96, 64)):
                        nc.vector.tensor_tensor(
                            t2[o:o + 32, :], tgt[sp:sp + 32, mo, NPREF:T],
                            sinT2n[sp:sp + 32, :], MUL)
                    nc.vector.tensor_tensor(src, t1[:], t2[:], ADD)

                def emit_attn(h, pump=None):
                    ph = (h % 2) * 64
                    kq = h // 2
                    po = ps_o.tile([128, 1024], f32, tag="ps_o",
                                   name=f"po_{bi}_{h}")

                    sq, sk = stacks[kq][h % 2]

                    def s_mm(ji):
                        j0, jw = TOK_TILES[ji]
                        pss = ps_s.tile([128, 1024], f32, tag="ps_s")
                        for q0 in range(0, 1024, 256):
                            nc.tensor.matmul(
                                pss[:jw, q0:q0 + 256],
                                sk[:, None, j0:j0 + jw].to_broadcast(
                                    (128, 2, jw)),
                                sq[:, :, q0:q0 + 256],
                                start=True, stop=True, perf_mode=DR)
                        return pss

                    # software pipeline: S(ji+1) and pump work sit between
                    # exp(ji) and AV(ji) in the in-order PE stream, so the PE
                    # computes next-tile scores while ACT runs exp(ji)
                    pss = s_mm(0)
                    for ji, (j0, jw) in enumerate(TOK_TILES):
                        first, last = ji == 0, ji == NJT - 1
                        es = apool.tile([128, 1024], bf16, tag="expS")
                        nc.scalar.activation(es[:jw, :], pss[:jw, :],
                                             FP.Exp, scale=SCALE)
                        if not last:
                            pss = s_mm(ji + 1)
                        if pump is not None:
                            pump()
                        for q0, qw in QCHUNKS:
                            nc.tensor.matmul(
                                po[:HD + 1, q0:q0 + qw],
                                Vst[:jw, ji, h, :],
                                es[:jw, q0:q0 + qw],
                                start=first, stop=last)
                    for qi, (q0, qw) in enumerate(QCHUNKS):
                        rc = apool2.tile([1, 512], f32, tag="recip")
                        nc.vector.reciprocal(rc[0:1, :qw], po[64:65, q0:q0 + qw])
                        rb = apool2.tile([64, 512], f32, tag="recipB")
                        nc.gpsimd.partition_broadcast(rb[:, :qw], rc[0:1, :qw])
                        nc.vector.tensor_tensor(
                            AOT[ph:ph + 64, kq, q0:q0 + qw],
                            po[0:64, q0:q0 + qw], rb[:, :qw], MUL)

                def emit_tail():
                    # 5-query tail for all 16 heads, batched: S packed into one
                    # ps_s slot (heads 0..10 bank A, 11..15 bank B), two exps,
                    # AV accumulated per head into one ps_o slot.
                    qt0, qtw = QTAIL
                    pst = ps_s.tile([128, 1024], f32, tag="ps_s",
                                    name=f"pst_{bi}")
                    nc.vector.memset(pst[:], 0.0)

                    def tcol(h):
                        return (h * qtw * NJT if h <= 10
                                else 512 + (h - 11) * qtw * NJT)

                    for h in range(NH):
                        ph = (h % 2) * 64
                        kq = h // 2
                        for ji, (j0, jw) in enumerate(TOK_TILES):
                            nc.tensor.matmul(
                                pst[:jw,
                                    tcol(h) + ji * qtw: tcol(h) + (ji + 1) * qtw],
                                KT[ph:ph + 64, kq, j0:j0 + jw],
                                QT[ph:ph + 64, kq, qt0:qt0 + qtw],
                                start=True, stop=True)
                    est = apool.tile([128, 1024], bf16, tag="expS",
                                     name=f"est_{bi}")
                    nc.scalar.activation(est[:, 0:495], pst[:, 0:495],
                                         FP.Exp, scale=SCALE)
                    nc.scalar.activation(est[:, 512:737], pst[:, 512:737],
                                         FP.Exp, scale=SCALE)
                    pot = ps_o.tile([128, 1024], f32, tag="ps_o",
                                    name=f"pot_{bi}")
                    for h in range(NH):
                        for ji, (j0, jw) in enumerate(TOK_TILES):
                            nc.tensor.matmul(
                                pot[:HD + 1, h * qtw:(h + 1) * qtw],
                                Vst[:jw, ji, h, :],
                                est[0:jw,
                                    tcol(h) + ji * qtw: tcol(h) + (ji + 1) * qtw],
                                start=(ji == 0), stop=(ji == NJT - 1))
                    rc = apool2.tile([1, 512], f32, tag="recip")
                    nc.vector.reciprocal(rc[0:1, :NH * qtw],
                                         pot[64:65, :NH * qtw])
                    rb = apool2.tile([64, 512], f32, tag="recipB")
                    nc.gpsimd.partition_broadcast(rb[:, :NH * qtw],
                                                  rc[0:1, :NH * qtw])
                    for h in range(NH):
                        nc.vector.tensor_tensor(
                            AOT[(h % 2) * 64:(h % 2) * 64 + 64, h // 2,
                                qt0:qt0 + qtw],
                            pot[0:64, h * qtw:(h + 1) * qtw],
                            rb[:, h * qtw:(h + 1) * qtw], MUL)

                def emit_outproj_g(ti, nci):
                    t0, tw = TOK_TILES[ti]
                    n0, nw = nci * 512, 512
                    pm = ps_w.tile([128, 512], f32, tag="ps_w",
                                   name=f"pmo_{bi}_{ti}_{n0}")
                    for ko in range(KO):
                        nc.tensor.matmul(
                            pm[:tw, :nw],
                            AOT[:, ko, t0:t0 + tw],
                            wb["wo"][:, ko, n0:n0 + nw],
                            start=(ko == 0), stop=False)
                    # K=1 ones-row matmul adds the bias; evac on ACT (the
                    # drain phase is DVE-bound, ACT idles there)
                    nc.tensor.matmul(
                        pm[:tw, :nw], ones_row[0:1, 0:tw],
                        bo_row[0:1, n0:n0 + nw], start=False, stop=True)
                    y = ypool.tile([128, 512], f32, tag="y")
                    nc.scalar.copy(y[:tw, :nw], pm[:tw, :nw])
                    nc.sync.dma_start(
                        out_d[tok0 + t0: tok0 + t0 + tw, n0:n0 + nw],
                        y[:tw, :nw])

                def emit_outproj(skip=()):
                    for ti in range(NJT):
                        for nci in range(2):
                            if (ti, nci) not in skip:
                                emit_outproj_g(ti, nci)

                def emit_blocks(extra=None):
                    # two queues: crit (next mo's q/k proj + rope — MUST be
                    # emitted within this mo's 18 pump slots: program order is
                    # semantics) and bulk (V chunks, deferred out-proj, xprep
                    # — emitted whenever slots are free, may spill)
                    crit = []
                    bulk = []

                    def pump():
                        if crit:
                            crit.pop(0)()
                        elif bulk:
                            bulk.pop(0)()

                    for mo in range(KO):
                        if mo < KO - 1:
                            crit.extend(
                                (lambda mo=mo, which=which, ci=ci:
                                 emit_qkproj_g(mo + 1, which, ci))
                                for which in range(2)
                                for ci in range(len(DRQ_CHUNKS)))
                            crit.append(lambda mo=mo: emit_rope_t(mo + 1, 0))
                            crit.append(lambda mo=mo: emit_rope_t(mo + 1, 1))
                            crit.append(lambda mo=mo: emit_stack(mo + 1))
                        if mo == 0:
                            bulk.extend(
                                (lambda ti=ti: emit_vproj_t(2, ti))
                                for ti in range(NJT))
                        if mo == 1:
                            bulk.extend(
                                (lambda ti=ti: emit_vproj_t(3, ti))
                                for ti in range(NJT))
                        if extra and mo in extra:
                            bulk.extend(extra[mo])
                        emit_attn(2 * mo, pump)
                        emit_attn(2 * mo + 1, pump)
                    while crit or bulk:
                        pump()

                def emit_head():
                    emit_vinit()
                    for ci in range(2):
                        for ti in range(NJT):
                            emit_vproj_t(ci, ti)
                    for which in range(2):
                        for ci in range(len(DRQ_CHUNKS)):
                            emit_qkproj_g(0, which, ci)
                    emit_rope_t(0, 0)
                    emit_rope_t(0, 1)
                    emit_stack(0)

                return {
                    "head": emit_head, "blocks": emit_blocks,
                    "tail": emit_tail, "outproj": emit_outproj,
                    "outproj_g": emit_outproj_g,
                }

            it0 = make_item(0, X8_0, XR_0)
            it0["head"]()
            X8_1 = ipool.tile([128, KO, TPAD], fp8, tag="X8", name="X8_1")
            XR_1 = ipool.tile([128, KO, TPAD], fp8, tag="XR", name="XR_1")
            it0["blocks"](extra={6: [lambda: emit_xprep_full(1, X8_1, XR_1)]})
            it0["tail"]()
            it1 = make_item(1, X8_1, XR_1)
            it1["head"]()              # runs during item0 out-proj
            # spread most of item0's out-proj through item1's attention as
            # extra pump work (2 chunks per mo)
            defer = [(ti, nci) for ti in range(1, NJT) for nci in range(2)]
            it0["outproj"](skip=defer)
            dthunks = [(lambda ti=ti, nci=nci: it0["outproj_g"](ti, nci))
                       for (ti, nci) in defer]
            extra = {mo: dthunks[mo * 2: mo * 2 + 2] for mo in range(7)}
            extra[7] = dthunks[14:]
            it1["blocks"](extra=extra)
            it1["tail"]()
            it1["outproj"]()

    nc.compile()
    return nc


_NC_CACHE = []
_LAST_RESULT = []


def _e4m3(x):
    return np.ascontiguousarray(x.astype(ml_dtypes.float8_e4m3))


def _split8(x):
    """x (f32) -> (hi, lo) e4m3 pair with hi + lo ~ x."""
    hi = x.astype(ml_dtypes.float8_e4m3)
    lo = (x - hi.astype(np.float32)).astype(ml_dtypes.float8_e4m3)
    return np.ascontiguousarray(hi), np.ascontiguousarray(lo)


def kernel(hidden_states, cos, sin, wq, bq, wk, wv, bv, wo, bo):
    from concourse.bass_utils import run_bass_kernel_spmd

    def _bf16(x):
        return np.ascontiguousarray(np.asarray(x).astype(ml_dtypes.bfloat16))

    def _f32(x):
        return np.ascontiguousarray(np.asarray(x, dtype=np.float32))

    hsT = np.asarray(hidden_states, dtype=np.float32).reshape(B * T, H).T
    hs8, hsr = _split8(np.ascontiguousarray(hsT))
    cos_t = np.asarray(cos, np.float32).T            # [64, 1024]
    sin_t = np.asarray(sin, np.float32).T
    # sign table indexed by SOURCE row (verifier requires same partitions for
    # both TT inputs): rows 0:32 = +sin[32:64], rows 32:64 = -sin[0:32]
    sin_n = np.concatenate([sin_t[32:64], -sin_t[0:32]], axis=0)
    shared = {
        "cosT2": _bf16(np.concatenate([cos_t, cos_t], axis=0)),
        "sinT2n": _bf16(np.concatenate([sin_n, sin_n], axis=0)),
        "wo": _bf16(wo),
        "bq": _f32(bq), "bv": _bf16(bv), "bo": _bf16(bo),
    }
    for wn, w in (("wq", wq), ("wk", wk), ("wv", wv)):
        hi, lo = _split8(np.asarray(w, dtype=np.float32) * WSC)
        shared[wn + "8"] = hi
        shared[wn + "r"] = lo
    if not _NC_CACHE:
        _NC_CACHE.append(build())
    nc = _NC_CACHE[0]

    in_maps = []
    for c in range(NCORES):
        m = dict(shared)
        m["hs8"] = np.ascontiguousarray(hs8[:, c * TOK:(c + 1) * TOK])
        m["hsr"] = np.ascontiguousarray(hsr[:, c * TOK:(c + 1) * TOK])
        in_maps.append(m)

    try:
        res = run_bass_kernel_spmd(nc, in_maps, core_ids=list(range(NCORES)))
    except Exception:
        # transient NRT device errors (e.g. NRT_EXEC_UNIT_UNRECOVERABLE) have
        # been observed on this fabric; one retry usually succeeds
        time.sleep(2.0)
        res = run_bass_kernel_spmd(nc, in_maps, core_ids=list(range(NCORES)))
    _LAST_RESULT.clear()
    _LAST_RESULT.append(res)
    out = np.concatenate(
        [r["out"].reshape(BPC, T, H) for r in res.results], axis=0)
    return out
